# revision 5
# baseline (speedup 1.0000x reference)
"""Segment-mean (word pooling) kernel for Trainium2, 8-core data parallel.

Computes, per batch row b:
  out[w, b, :] = mean of char_feats[l, b, :] over positions l where
                 word_ids[b, l] == w and l is a valid (non [CLS]/[SEP]) char,
  zeroed for w >= word_num[b];  plus the [W, B] word mask.

Device strategy (per core, B/8 = 32 batch rows):
  - one-hot selector S[p, w] = (seg[b, 128k+p] == w) built on VectorE
    (tensor_scalar is_equal against an iota row),
  - seg_sum accumulated on TensorE: PSUM[w, d] += S^T @ feats_chunk,
  - ScalarE applies the per-word scale (mask/count, host-precomputed
    [128, B] f32) while copying PSUM -> SBUF,
  - 1MB streaming DMAs for feats in / word_feats out.
"""

import numpy as np

_L, _B, _D, _W = 512, 256, 256, 128
_NC = 8
_BL = _B // _NC  # 32 batch rows per core
_KC = _L // 128  # 4 contraction chunks
_GB = 8          # batch rows per DMA group (1MB tiles)

_progs = {}


def _build_program(iters=1):
    """Build the per-core Bass program. iters>1 repeats the whole body
    (same inputs/outputs) for device-time benchmarking via timing deltas."""
    if iters in _progs:
        return _progs[iters]

    import concourse.bass as bass
    import concourse.tile as tile
    from concourse import bacc, mybir

    f32 = mybir.dt.float32
    nc = bacc.Bacc("TRN2", target_bir_lowering=False, debug=False)

    feats = nc.dram_tensor("feats", [_L, _BL, _D], f32, kind="ExternalInput").ap()
    segf = nc.dram_tensor("segf", [128, _BL, _KC], f32, kind="ExternalInput").ap()
    scale = nc.dram_tensor("scale", [128, _BL], f32, kind="ExternalInput").ap()
    out = nc.dram_tensor("out", [_W, _BL, _D], f32, kind="ExternalOutput").ap()

    with tile.TileContext(nc) as tc:
        with (
            tc.tile_pool(name="const", bufs=1) as constp,
            tc.tile_pool(name="fpool", bufs=2) as fpool,
            tc.tile_pool(name="spool", bufs=3) as spool,
            tc.tile_pool(name="opool", bufs=2) as opool,
            tc.tile_pool(name="psum", bufs=4, space=bass.MemorySpace.PSUM) as ppool,
        ):
            iota_t = constp.tile([128, _W], f32)
            nc.gpsimd.iota(
                iota_t[:],
                pattern=[[1, _W]],
                base=0,
                channel_multiplier=0,
                allow_small_or_imprecise_dtypes=True,
            )
            segf_t = constp.tile([128, _BL, _KC], f32)
            nc.sync.dma_start(segf_t[:], segf[:])
            scale_t = constp.tile([128, _BL], f32)
            nc.sync.dma_start(scale_t[:], scale[:])

            for g in range(iters * (_BL // _GB)):
                g = g % (_BL // _GB)
                ftiles = []
                for k in range(_KC):
                    ft = fpool.tile([128, _GB, _D], f32, tag=f"ft{k}")
                    nc.sync.dma_start(
                        ft[:], feats[k * 128 : (k + 1) * 128, g * _GB : (g + 1) * _GB, :]
                    )
                    ftiles.append(ft)
                ob = opool.tile([128, _GB, _D], f32)
                for bi in range(_GB):
                    b = g * _GB + bi
                    s_t = spool.tile([128, _KC * 128], f32)
                    for k in range(_KC):
                        nc.vector.tensor_scalar(
                            out=s_t[:, k * 128 : (k + 1) * 128],
                            in0=iota_t[:],
                            scalar1=segf_t[:, b, k : k + 1],
                            scalar2=None,
                            op0=mybir.AluOpType.is_equal,
                        )
                    ps = ppool.tile([_W, _D], f32)
                    for k in range(_KC):
                        nc.tensor.matmul(
                            ps[:],
                            s_t[:, k * 128 : (k + 1) * 128],
                            ftiles[k][:, bi, :],
                            start=(k == 0),
                            stop=(k == _KC - 1),
                        )
                    nc.scalar.activation(
                        ob[:, bi, :],
                        ps[:],
                        mybir.ActivationFunctionType.Copy,
                        scale=scale_t[:, b : b + 1],
                    )
                # store from ACT's own HWDGE ring (qActDynamicHW) so the
                # compute-gated store never head-of-line blocks the input
                # stream on sync's ring (qSPDynamicHW)
                nc.scalar.dma_start(out[:, g * _GB : (g + 1) * _GB, :], ob[:])

    nc.compile()
    _progs[iters] = nc
    return nc


def _host_prep(char_feats, word_ids, attention_mask):
    """Per-word scalars (tiny, data-dependent) computed on host."""
    B, L = word_ids.shape
    W = _W
    char_nums = attention_mask.astype(np.int64).sum(axis=1) - 2
    pos = np.arange(L)
    valid = (pos[None, :] >= 1) & (pos[None, :] < 1 + char_nums[:, None])
    seg = np.where(valid, word_ids, W).astype(np.int64)  # [B, L], W = overflow
    flat = seg + (W + 1) * np.arange(B)[:, None]
    counts = (
        np.bincount(flat.ravel(), minlength=B * (W + 1))
        .reshape(B, W + 1)[:, :W]
        .astype(np.float32)
    )  # [B, W]
    word_nums = word_ids.max(axis=1) + 1
    masks = np.arange(W)[:, None] < word_nums[None, :]  # [W, B] bool
    scale = masks.astype(np.float32) / np.maximum(counts.T, 1.0)  # [W, B]
    # seg values laid out for SBUF: segf[p, b, k] = seg[b, 128k + p]
    segf = np.ascontiguousarray(
        seg.reshape(B, _KC, 128).transpose(2, 0, 1).astype(np.float32)
    )  # [128, B, KC]
    return segf, scale, masks


def kernel(char_feats, word_ids, attention_mask):
    from concourse.bass_utils import run_bass_kernel_spmd

    nc = _build_program()
    segf, scale, masks = _host_prep(
        np.asarray(char_feats), np.asarray(word_ids), np.asarray(attention_mask)
    )
    cf = np.asarray(char_feats, dtype=np.float32)

    in_maps = []
    for c in range(_NC):
        sl = slice(c * _BL, (c + 1) * _BL)
        in_maps.append(
            {
                "feats": np.ascontiguousarray(cf[:, sl, :]),
                "segf": np.ascontiguousarray(segf[:, sl, :]),
                "scale": np.ascontiguousarray(scale[:, sl]),
            }
        )

    res = run_bass_kernel_spmd(nc, in_maps, list(range(_NC)))
    word_feats = np.concatenate(
        [res.results[c]["out"] for c in range(_NC)], axis=1
    )  # [W, B, D]
    return word_feats, masks


# revision 8
# speedup vs baseline: 6.0608x; 6.0608x over previous
"""Segment-mean (word pooling) kernel for Trainium2, 8-core data parallel.

Computes, per batch row b:
  out[w, b, :] = mean of char_feats[l, b, :] over positions l where
                 word_ids[b, l] == w and l is a valid (non [CLS]/[SEP]) char,
  zeroed for w >= word_num[b];  plus the [W, B] word mask.

Device strategy (per core, B/8 = 32 batch rows):
  - one-hot selector S[p, w] = (seg[b, 128k+p] == w) built on VectorE
    (tensor_scalar is_equal against an iota row),
  - seg_sum accumulated on TensorE: PSUM[w, d] += S^T @ feats_chunk,
  - ScalarE applies the per-word scale (mask/count, host-precomputed
    [128, B] f32) while copying PSUM -> SBUF,
  - 1MB streaming DMAs for feats in / word_feats out.
"""

import numpy as np

_L, _B, _D, _W = 512, 256, 256, 128
_NC = 8
_BL = _B // _NC  # 32 batch rows per core
_KC = _L // 128  # 4 contraction chunks
_GB = 8          # batch rows per DMA group (1MB tiles)

_progs = {}


def _build_program(iters=1):
    """Build the per-core Bass program. iters>1 repeats the whole body
    (same inputs/outputs) for device-time benchmarking via timing deltas."""
    if iters in _progs:
        return _progs[iters]

    import concourse.bass as bass
    import concourse.tile as tile
    from concourse import bacc, mybir

    f32 = mybir.dt.float32
    nc = bacc.Bacc("TRN2", target_bir_lowering=False, debug=False)

    feats = nc.dram_tensor("feats", [_L, _BL, _D], f32, kind="ExternalInput").ap()
    segf = nc.dram_tensor("segf", [128, _BL, _KC], f32, kind="ExternalInput").ap()
    scale = nc.dram_tensor("scale", [128, _BL], f32, kind="ExternalInput").ap()
    out = nc.dram_tensor("out", [_W, _BL, _D], f32, kind="ExternalOutput").ap()

    with tile.TileContext(nc) as tc:
        with (
            tc.tile_pool(name="const", bufs=1) as constp,
            tc.tile_pool(name="fpool", bufs=2) as fpool,
            tc.tile_pool(name="spool", bufs=3) as spool,
            tc.tile_pool(name="opool", bufs=2) as opool,
            tc.tile_pool(name="psum", bufs=4, space=bass.MemorySpace.PSUM) as ppool,
        ):
            iota_t = constp.tile([128, _W], f32)
            nc.gpsimd.iota(
                iota_t[:],
                pattern=[[1, _W]],
                base=0,
                channel_multiplier=0,
                allow_small_or_imprecise_dtypes=True,
            )
            segf_t = constp.tile([128, _BL, _KC], f32)
            nc.sync.dma_start(segf_t[:], segf[:])
            scale_t = constp.tile([128, _BL], f32)
            nc.sync.dma_start(scale_t[:], scale[:])

            import contextlib

            loop_cm = (
                tc.For_i(0, iters, 1, hint_engines=(mybir.EngineType.PE,))
                if iters > 1
                else contextlib.nullcontext()
            )
            with loop_cm:
                for g in range(_BL // _GB):
                    ftiles = []
                    for k in range(_KC):
                        ft = fpool.tile([128, _GB, _D], f32, tag=f"ft{k}")
                        eng = nc.sync if k % 2 == 0 else nc.gpsimd
                        eng.dma_start(
                            ft[:],
                            feats[k * 128 : (k + 1) * 128, g * _GB : (g + 1) * _GB, :],
                        )
                        ftiles.append(ft)
                    ob = opool.tile([128, _GB, _D], f32)
                    for bi in range(_GB):
                        b = g * _GB + bi
                        s_t = spool.tile([128, _KC * 128], f32)
                        for k in range(_KC):
                            nc.vector.tensor_scalar(
                                out=s_t[:, k * 128 : (k + 1) * 128],
                                in0=iota_t[:],
                                scalar1=segf_t[:, b, k : k + 1],
                                scalar2=None,
                                op0=mybir.AluOpType.is_equal,
                            )
                        ps = ppool.tile([_W, _D], f32)
                        for k in range(_KC):
                            nc.tensor.matmul(
                                ps[:],
                                s_t[:, k * 128 : (k + 1) * 128],
                                ftiles[k][:, bi, :],
                                start=(k == 0),
                                stop=(k == _KC - 1),
                            )
                        nc.scalar.activation(
                            ob[:, bi, :],
                            ps[:],
                            mybir.ActivationFunctionType.Copy,
                            scale=scale_t[:, b : b + 1],
                        )
                    # store from ACT's own HWDGE ring (qActDynamicHW) so the
                    # compute-gated store never head-of-line blocks the input
                    # stream on sync's ring (qSPDynamicHW)
                    nc.scalar.dma_start(out[:, g * _GB : (g + 1) * _GB, :], ob[:])

    nc.compile()
    _progs[iters] = nc
    return nc


def _host_prep(char_feats, word_ids, attention_mask):
    """Per-word scalars (tiny, data-dependent) computed on host."""
    B, L = word_ids.shape
    W = _W
    char_nums = attention_mask.astype(np.int64).sum(axis=1) - 2
    pos = np.arange(L)
    valid = (pos[None, :] >= 1) & (pos[None, :] < 1 + char_nums[:, None])
    seg = np.where(valid, word_ids, W).astype(np.int64)  # [B, L], W = overflow
    flat = seg + (W + 1) * np.arange(B)[:, None]
    counts = (
        np.bincount(flat.ravel(), minlength=B * (W + 1))
        .reshape(B, W + 1)[:, :W]
        .astype(np.float32)
    )  # [B, W]
    word_nums = word_ids.max(axis=1) + 1
    masks = np.arange(W)[:, None] < word_nums[None, :]  # [W, B] bool
    scale = masks.astype(np.float32) / np.maximum(counts.T, 1.0)  # [W, B]
    # seg values laid out for SBUF: segf[p, b, k] = seg[b, 128k + p]
    segf = np.ascontiguousarray(
        seg.reshape(B, _KC, 128).transpose(2, 0, 1).astype(np.float32)
    )  # [128, B, KC]
    return segf, scale, masks


def kernel(char_feats, word_ids, attention_mask):
    from concourse.bass_utils import run_bass_kernel_spmd

    nc = _build_program()
    segf, scale, masks = _host_prep(
        np.asarray(char_feats), np.asarray(word_ids), np.asarray(attention_mask)
    )
    cf = np.asarray(char_feats, dtype=np.float32)

    in_maps = []
    for c in range(_NC):
        sl = slice(c * _BL, (c + 1) * _BL)
        in_maps.append(
            {
                "feats": np.ascontiguousarray(cf[:, sl, :]),
                "segf": np.ascontiguousarray(segf[:, sl, :]),
                "scale": np.ascontiguousarray(scale[:, sl]),
            }
        )

    res = run_bass_kernel_spmd(nc, in_maps, list(range(_NC)))
    word_feats = np.concatenate(
        [res.results[c]["out"] for c in range(_NC)], axis=1
    )  # [W, B, D]
    return word_feats, masks


# revision 9
# speedup vs baseline: 10.7304x; 1.7705x over previous
"""Segment-mean (word pooling) kernel for Trainium2, 8-core data parallel.

Computes, per batch row b:
  out[w, b, :] = mean of char_feats[l, b, :] over positions l where
                 word_ids[b, l] == w and l is a valid (non [CLS]/[SEP]) char,
  zeroed for w >= word_num[b];  plus the [W, B] word mask.

Device strategy (per core, B/8 = 32 batch rows):
  - one-hot selector S[p, w] = (seg[b, 128k+p] == w) built on VectorE
    (tensor_scalar is_equal against an iota row),
  - seg_sum accumulated on TensorE: PSUM[w, d] += S^T @ feats_chunk,
  - ScalarE applies the per-word scale (mask/count, host-precomputed
    [128, B] f32) while copying PSUM -> SBUF,
  - 1MB streaming DMAs for feats in / word_feats out.
"""

import numpy as np

_L, _B, _D, _W = 512, 256, 256, 128
_NC = 8
_BL = _B // _NC  # 32 batch rows per core
_KC = _L // 128  # 4 contraction chunks
_GB = 8          # batch rows per DMA group (1MB tiles)

_progs = {}


def _build_program(iters=1):
    """Build the per-core Bass program. iters>1 repeats the whole body
    (same inputs/outputs) for device-time benchmarking via timing deltas."""
    if iters in _progs:
        return _progs[iters]

    import concourse.bass as bass
    import concourse.tile as tile
    from concourse import bacc, mybir

    f32 = mybir.dt.float32
    nc = bacc.Bacc("TRN2", target_bir_lowering=False, debug=False)

    feats = nc.dram_tensor("feats", [_L, _BL, _D], f32, kind="ExternalInput").ap()
    segf = nc.dram_tensor("segf", [128, _BL, _KC], f32, kind="ExternalInput").ap()
    scale = nc.dram_tensor("scale", [128, _BL], f32, kind="ExternalInput").ap()
    out = nc.dram_tensor("out", [_W, _BL, _D], f32, kind="ExternalOutput").ap()

    with tile.TileContext(nc) as tc:
        with (
            tc.tile_pool(name="const", bufs=1) as constp,
            tc.tile_pool(name="fpool", bufs=3) as fpool,
            tc.tile_pool(name="spool", bufs=3) as spool,
            tc.tile_pool(name="opool", bufs=2) as opool,
            tc.tile_pool(name="psum", bufs=4, space=bass.MemorySpace.PSUM) as ppool,
        ):
            iota_t = constp.tile([128, _W], f32)
            nc.gpsimd.iota(
                iota_t[:],
                pattern=[[1, _W]],
                base=0,
                channel_multiplier=0,
                allow_small_or_imprecise_dtypes=True,
            )
            segf_t = constp.tile([128, _BL, _KC], f32)
            nc.sync.dma_start(segf_t[:], segf[:])
            scale_t = constp.tile([128, _BL], f32)
            nc.sync.dma_start(scale_t[:], scale[:])

            import contextlib

            loop_cm = (
                tc.For_i(0, iters, 1, hint_engines=(mybir.EngineType.PE,))
                if iters > 1
                else contextlib.nullcontext()
            )
            with loop_cm:
                for g in range(_BL // _GB):
                    ftiles = []
                    for k in range(_KC):
                        ft = fpool.tile([128, _GB, _D], f32, tag=f"ft{k}")
                        eng = nc.sync if k % 2 == 0 else nc.gpsimd
                        eng.dma_start(
                            ft[:],
                            feats[k * 128 : (k + 1) * 128, g * _GB : (g + 1) * _GB, :],
                        )
                        ftiles.append(ft)
                    ob = opool.tile([128, _GB, _D], f32)
                    for bi in range(_GB):
                        b = g * _GB + bi
                        s_t = spool.tile([128, _KC * 128], f32)
                        for k in range(_KC):
                            nc.vector.tensor_scalar(
                                out=s_t[:, k * 128 : (k + 1) * 128],
                                in0=iota_t[:],
                                scalar1=segf_t[:, b, k : k + 1],
                                scalar2=None,
                                op0=mybir.AluOpType.is_equal,
                            )
                        ps = ppool.tile([_W, _D], f32)
                        for k in range(_KC):
                            nc.tensor.matmul(
                                ps[:],
                                s_t[:, k * 128 : (k + 1) * 128],
                                ftiles[k][:, bi, :],
                                start=(k == 0),
                                stop=(k == _KC - 1),
                            )
                        nc.scalar.activation(
                            ob[:, bi, :],
                            ps[:],
                            mybir.ActivationFunctionType.Copy,
                            scale=scale_t[:, b : b + 1],
                        )
                    # store from ACT's own HWDGE ring (qActDynamicHW) so the
                    # compute-gated store never head-of-line blocks the input
                    # stream on sync's ring (qSPDynamicHW)
                    nc.scalar.dma_start(out[:, g * _GB : (g + 1) * _GB, :], ob[:])

    nc.compile()
    _progs[iters] = nc
    return nc


def _host_prep(char_feats, word_ids, attention_mask):
    """Per-word scalars (tiny, data-dependent) computed on host."""
    B, L = word_ids.shape
    W = _W
    char_nums = attention_mask.astype(np.int64).sum(axis=1) - 2
    pos = np.arange(L)
    valid = (pos[None, :] >= 1) & (pos[None, :] < 1 + char_nums[:, None])
    seg = np.where(valid, word_ids, W).astype(np.int64)  # [B, L], W = overflow
    flat = seg + (W + 1) * np.arange(B)[:, None]
    counts = (
        np.bincount(flat.ravel(), minlength=B * (W + 1))
        .reshape(B, W + 1)[:, :W]
        .astype(np.float32)
    )  # [B, W]
    word_nums = word_ids.max(axis=1) + 1
    masks = np.arange(W)[:, None] < word_nums[None, :]  # [W, B] bool
    scale = masks.astype(np.float32) / np.maximum(counts.T, 1.0)  # [W, B]
    # seg values laid out for SBUF: segf[p, b, k] = seg[b, 128k + p]
    segf = np.ascontiguousarray(
        seg.reshape(B, _KC, 128).transpose(2, 0, 1).astype(np.float32)
    )  # [128, B, KC]
    return segf, scale, masks


def kernel(char_feats, word_ids, attention_mask):
    from concourse.bass_utils import run_bass_kernel_spmd

    nc = _build_program()
    segf, scale, masks = _host_prep(
        np.asarray(char_feats), np.asarray(word_ids), np.asarray(attention_mask)
    )
    cf = np.asarray(char_feats, dtype=np.float32)

    in_maps = []
    for c in range(_NC):
        sl = slice(c * _BL, (c + 1) * _BL)
        in_maps.append(
            {
                "feats": np.ascontiguousarray(cf[:, sl, :]),
                "segf": np.ascontiguousarray(segf[:, sl, :]),
                "scale": np.ascontiguousarray(scale[:, sl]),
            }
        )

    res = run_bass_kernel_spmd(nc, in_maps, list(range(_NC)))
    word_feats = np.concatenate(
        [res.results[c]["out"] for c in range(_NC)], axis=1
    )  # [W, B, D]
    return word_feats, masks
